# revision 1
# baseline (speedup 1.0000x reference)
"""Trainium2 Bass kernel for nn_MultiHeadAttention_77412490543447.

reference:
  qkv = (x @ W_qkv + b_qkv) -> q,k,v  (B,H,S,D)
  S   = scale * (q k^T + einsum('xyc,bhxc->bhxy', pe, q))
  out = (S @ v) @ W_out + b_out

Sharding: query-position (x) blocks of 128 per core, 8 cores; k/v computed
fully on every core (v1); pe sharded by x, host pre-transposed to [x, c, y]
bf16.  Matmuls bf16 with fp32 PSUM accumulation; scale folded into W_q/b_q.
"""

import os
import numpy as np
import ml_dtypes

import concourse.bass as bass
import concourse.bacc as bacc
import concourse.mybir as mybir
import concourse.tile as tile
from concourse.bass_utils import run_bass_kernel_spmd

BF = mybir.dt.bfloat16
F32 = mybir.dt.float32
ADD = mybir.AluOpType.add

B, S, E = 4, 1024, 1024
H, D = 16, 64
NCORES = 8
XB = S // NCORES          # 128 query positions per core
TOK = B * S               # 4096 tokens
OWN = B * XB              # 512 own tokens
KC = E // 128             # 8 contraction chunks
FT = E // 128             # 8 feature tiles
HP = H // 2               # 8 head pairs
YC = S // 128             # 8 y chunks

_compiled = None
KPHASES = int(os.environ.get('KPHASES', '5'))


def build_kernel():
    nc = bacc.Bacc(None, target_bir_lowering=False)

    xT = nc.dram_tensor("xT", [E, TOK], BF, kind="ExternalInput")
    xTo = nc.dram_tensor("xTo", [E, OWN], BF, kind="ExternalInput")
    wq = nc.dram_tensor("wq", [E, E], BF, kind="ExternalInput")
    wk = nc.dram_tensor("wk", [E, E], BF, kind="ExternalInput")
    wv = nc.dram_tensor("wv", [E, E], BF, kind="ExternalInput")
    wo = nc.dram_tensor("wo", [E, E], BF, kind="ExternalInput")
    pet = nc.dram_tensor("pet", [XB // 2, 128, S], BF, kind="ExternalInput")
    bq = nc.dram_tensor("bq", [1, E], BF, kind="ExternalInput")
    bk = nc.dram_tensor("bk", [1, E], BF, kind="ExternalInput")
    bv = nc.dram_tensor("bv", [1, E], BF, kind="ExternalInput")
    bo = nc.dram_tensor("bo", [1, E], BF, kind="ExternalInput")
    out = nc.dram_tensor("out", [OWN, E], F32, kind="ExternalOutput")

    with tile.TileContext(nc) as tc:
        with (
            tc.tile_pool(name="dram", bufs=1, space="DRAM") as dram,
            tc.tile_pool(name="const", bufs=1) as const,
            tc.tile_pool(name="resident", bufs=1) as res,
            tc.tile_pool(name="stage", bufs=6) as stage,
            tc.tile_pool(name="ps", bufs=5, space="PSUM") as psA,
            tc.tile_pool(name="psacc", bufs=3, space="PSUM") as psAcc,
        ):
            kdram = dram.tile([FT, 128, TOK], BF)          # k^T (ft, c, tok)
            vdram = dram.tile([TOK // 128, 128, E], BF)    # v   (tt, row, feat)

            ones = const.tile([1, 512], BF)
            nc.vector.memset(ones[:], 1.0)
            bq_sb = const.tile([1, E], BF, tag="bq")
            bk_sb = const.tile([1, E], BF, tag="bk")
            bv_sb = const.tile([1, E], BF, tag="bv")
            bo_sb = const.tile([1, E], BF, tag="bo")
            nc.sync.dma_start(bq_sb[:], bq[:])
            nc.sync.dma_start(bk_sb[:], bk[:])
            nc.sync.dma_start(bv_sb[:], bv[:])
            nc.sync.dma_start(bo_sb[:], bo[:])

            # qP: [128=(dup*64+c), x, h, b], dup halves identical (bias incl.)
            qP = res.tile([128, XB, H, B], BF, tag="qP")
            # attnT: [128=(par*64+d), hp, b, x] bf16 (psum evicts cast here)
            attnT_bf = res.tile([128, HP, B, XB], BF, tag="attnT_bf")

            # ---------------- projections ----------------
            with tc.tile_pool(name="proj", bufs=1) as proj:
                xT_sb = proj.tile([128, KC, TOK], BF, tag="xT")
                for kc in range(KC):
                    nc.sync.dma_start(xT_sb[:, kc, :], xT[kc * 128:(kc + 1) * 128, :])

                # k-proj -> kdram
                wk_sb = proj.tile([128, KC, E], BF, tag="wk")
                for kc in range(KC):
                    nc.sync.dma_start(wk_sb[:, kc, :], wk[kc * 128:(kc + 1) * 128, :])
                for ft in range(FT):
                    for nt in range(TOK // 512):
                        ps = psA.tile([128, 512], F32, tag="ps")
                        for kc in range(KC):
                            nc.tensor.matmul(
                                ps[:],
                                wk_sb[:, kc, ft * 128:(ft + 1) * 128],
                                xT_sb[:, kc, nt * 512:(nt + 1) * 512],
                                start=(kc == 0), stop=False,
                            )
                        nc.tensor.matmul(   # + b_k (per partition row)
                            ps[:], bk_sb[:, ft * 128:(ft + 1) * 128],
                            ones[:, :512], start=False, stop=True,
                        )
                        st = stage.tile([128, 512], BF, tag="st")
                        nc.scalar.copy(st[:], ps[:])
                        nc.gpsimd.dma_start(
                            kdram[ft, :, nt * 512:(nt + 1) * 512], st[:])

                # v-proj -> vdram
                wv_sb = proj.tile([128, KC, E], BF, tag="wk")
                for kc in range(KC):
                    nc.sync.dma_start(wv_sb[:, kc, :], wv[kc * 128:(kc + 1) * 128, :])
                for tt in range(TOK // 128):
                    for n2 in range(2):
                        ps = psA.tile([128, 512], F32, tag="ps")
                        for kc in range(KC):
                            nc.tensor.matmul(
                                ps[:],
                                xT_sb[:, kc, tt * 128:(tt + 1) * 128],
                                wv_sb[:, kc, n2 * 512:(n2 + 1) * 512],
                                start=(kc == 0), stop=False,
                            )
                        nc.tensor.matmul(   # + b_v (free-dim broadcast)
                            ps[:], ones[:, :128],
                            bv_sb[:, n2 * 512:(n2 + 1) * 512],
                            start=False, stop=True,
                        )
                        st = stage.tile([128, 512], BF, tag="st")
                        nc.scalar.copy(st[:], ps[:])
                        nc.gpsimd.dma_start(
                            vdram[tt, :, n2 * 512:(n2 + 1) * 512], st[:])

                # q-proj (own tokens; scale folded into wq/bq)
                wq_sb = proj.tile([128, KC, E], BF, tag="wk")
                for kc in range(KC):
                    nc.sync.dma_start(wq_sb[:, kc, :], wq[kc * 128:(kc + 1) * 128, :])
                xTo_sb = proj.tile([128, KC, OWN], BF, tag="xTo")
                for kc in range(KC):
                    nc.sync.dma_start(xTo_sb[:, kc, :], xTo[kc * 128:(kc + 1) * 128, :])

                for hp in range(HP):
                    ps = psA.tile([128, 512], F32, tag="ps")
                    for par in range(2):
                        h = 2 * hp + par
                        for kc in range(KC):
                            nc.tensor.matmul(
                                ps[par * 64:(par + 1) * 64, :],
                                wq_sb[:, kc, h * 64:(h + 1) * 64],
                                xTo_sb[:, kc, :],
                                start=(kc == 0), stop=False,
                                tile_position=(0, par * 64),
                                skip_group_check=True,
                            )
                        nc.tensor.matmul(   # + b_q rows for this head
                            ps[par * 64:(par + 1) * 64, :],
                            bq_sb[:, h * 64:(h + 1) * 64],
                            ones[:, :512],
                            start=False, stop=True,
                            tile_position=(0, par * 64),
                            skip_group_check=True,
                        )
                    # psum free order (b, x); qP free (x,h,b)
                    for par in range(2):
                        h = 2 * hp + par
                        for dup in range(2):
                            dst = qP[dup * 64:(dup + 1) * 64, :, h, :].rearrange(
                                "c x b -> c b x")
                            nc.scalar.copy(
                                dst, ps[par * 64:(par + 1) * 64, :])

            # ---------------- attention ----------------
            # S: [128=y, yc, x, h, b] bf16
            with tc.tile_pool(name="attnS", bufs=1) as attnS:
              S_t = [attnS.tile([128, XB, H, B], BF, tag=f"S{i}", name=f"S{i}")
                     for i in range(YC)]

              # pe part: groups of 8 x (4 pairs) per psum bank
              with tc.tile_pool(name="pe", bufs=10) as pe_pool:
                  for xg in range(XB // 8 if KPHASES >= 2 else 0):
                      pts = []
                      for p in range(4):
                          pt = pe_pool.tile([128, S], BF, tag="pet")
                          nc.sync.dma_start(pt[:], pet[xg * 4 + p, :, :])
                          pts.append(pt)
                      for yc in range(YC):
                          pse = psA.tile([128, 512], F32, tag="ps", name=f"pse{xg}_{yc}")
                          pso = psA.tile([128, 512], F32, tag="ps", name=f"pso{xg}_{yc}")
                          for p in range(4):
                              for xpar in range(2):
                                  x = xg * 8 + 2 * p + xpar
                                  tgt = pse if xpar == 0 else pso
                                  nc.tensor.matmul(
                                      tgt[:, p * 64:(p + 1) * 64],
                                      pts[p][xpar * 64:(xpar + 1) * 64,
                                             yc * 128:(yc + 1) * 128],
                                      qP[xpar * 64:(xpar + 1) * 64, x, :, :],
                                      start=True, stop=True,
                                      tile_position=(xpar * 64, 0),
                                      skip_group_check=True,
                                  )
                          sv = S_t[yc][:, xg * 8:(xg + 1) * 8, :, :].rearrange(
                              "p (q xp) h b -> p xp q h b", xp=2)
                          nc.scalar.copy(sv[:, 0], pse[:, :256])
                          nc.vector.tensor_copy(sv[:, 1], pso[:, :256])

              # k part: S[yc,:,h,b] += (kT slice)^T @ qP ; 4 heads per psum tile
              with tc.tile_pool(name="kslab", bufs=3) as kslab_pool:
                  for b in range(B if KPHASES >= 3 else 0):
                      for yc in range(YC):
                          ksl = kslab_pool.tile([128, FT, 128], BF, tag="ksl")
                          t0 = b * S + yc * 128
                          nc.gpsimd.dma_start(
                              ksl[:],
                              kdram[:, :, t0:t0 + 128].rearrange("f c y -> c f y"))
                          for par in range(2):
                              for qd in range(2):      # same-parity head quads
                                  ps = psA.tile([128, 512], F32, tag="ps",
                                                name=f"kp{b}_{yc}_{par}_{qd}")
                                  for i in range(4):
                                      hh = qd * 4 + i
                                      h = 2 * hh + par
                                      nc.tensor.matmul(
                                          ps[:, i * 128:(i + 1) * 128],
                                          ksl[par * 64:(par + 1) * 64, h // 2, :],
                                          qP[par * 64:(par + 1) * 64, :, h, b],
                                          start=True, stop=True,
                                          tile_position=(par * 64, 0),
                                          skip_group_check=True,
                                      )
                                  dst = S_t[yc][:, :, :, b].rearrange(
                                      "p x (hh hpar) -> p hpar hh x", hpar=2)[
                                      :, par, qd * 4:(qd + 1) * 4, :]
                                  nc.vector.tensor_tensor(dst, ps[:], dst, ADD)

              # attn = S @ v accumulated over yc; attnT[d, x] per (hp, b)
              with tc.tile_pool(name="vslab", bufs=3) as vslab_pool:
                  for b in range(B if KPHASES >= 4 else 0):
                      acc = [psAcc.tile([128, 512], F32, tag="acc", name=f"acc{b}_{i}") for i in range(2)]
                      for yc in range(YC):
                          vsl = vslab_pool.tile([128, E], BF, tag="vsl")
                          nc.gpsimd.dma_start(vsl[:], vdram[b * 8 + yc, :, :])
                          for hp in range(HP):
                              for par in range(2):
                                  h = 2 * hp + par
                                  nc.tensor.matmul(
                                      acc[hp // 4][par * 64:(par + 1) * 64,
                                                   (hp % 4) * 128:(hp % 4 + 1) * 128],
                                      vsl[:, h * 64:(h + 1) * 64],
                                      S_t[yc][:, :, h, b],
                                      start=(yc == 0 and hp % 4 == 0),
                                      stop=(yc == YC - 1),
                                      tile_position=(0, par * 64),
                                      skip_group_check=True,
                                  )
                      nc.scalar.copy(attnT_bf[:, 0:4, b, :], acc[0][:])
                      nc.vector.tensor_copy(attnT_bf[:, 4:8, b, :], acc[1][:])

            # ---------------- output projection ----------------
            with tc.tile_pool(name="outp", bufs=1) as outp:
                wo_sb = outp.tile([128, KC, E], BF, tag="wo")
                for kc in range(KC if KPHASES >= 5 else 0):
                    nc.sync.dma_start(wo_sb[:, kc, :], wo[kc * 128:(kc + 1) * 128, :])
                for b in range(B if KPHASES >= 5 else 0):
                    for n2 in range(2):
                        ps = psA.tile([128, 512], F32, tag="ps")
                        for kc in range(KC):
                            nc.tensor.matmul(
                                ps[:],
                                attnT_bf[:, kc, b, :],
                                wo_sb[:, kc, n2 * 512:(n2 + 1) * 512],
                                start=(kc == 0), stop=False,
                            )
                        nc.tensor.matmul(
                            ps[:], ones[:, :128],
                            bo_sb[:, n2 * 512:(n2 + 1) * 512],
                            start=False, stop=True,
                        )
                        so = stage.tile([128, 512], F32, tag="so")
                        nc.scalar.copy(so[:], ps[:])
                        nc.sync.dma_start(
                            out[b * 128:(b + 1) * 128, n2 * 512:(n2 + 1) * 512],
                            so[:])
    nc.compile()
    return nc


def shard_inputs(x, W_qkv, b_qkv, pe, W_out, b_out):
    bf = ml_dtypes.bfloat16
    scale = D ** -0.5
    x2 = np.asarray(x, np.float32).reshape(TOK, E)
    xT = np.ascontiguousarray(x2.T).astype(bf)
    Wq = (np.asarray(W_qkv[:, :E], np.float32) * scale).astype(bf)
    Wk = np.asarray(W_qkv[:, E:2 * E], np.float32).astype(bf)
    Wv = np.asarray(W_qkv[:, 2 * E:], np.float32).astype(bf)
    Wo = np.asarray(W_out, np.float32).astype(bf)
    bqv = (np.asarray(b_qkv[:E], np.float32) * scale).astype(bf).reshape(1, E)
    bkv = np.asarray(b_qkv[E:2 * E], np.float32).astype(bf).reshape(1, E)
    bvv = np.asarray(b_qkv[2 * E:], np.float32).astype(bf).reshape(1, E)
    bov = np.asarray(b_out, np.float32).astype(bf).reshape(1, E)

    pe32 = np.asarray(pe, np.float32)
    in_maps = []
    for c in range(NCORES):
        x0 = c * XB
        pet_c = np.ascontiguousarray(
            pe32[x0:x0 + XB].transpose(0, 2, 1)).reshape(XB // 2, 128, S)
        cols = (np.arange(B)[:, None] * S + (x0 + np.arange(XB))[None, :]).ravel()
        xTo = np.ascontiguousarray(xT[:, cols])
        in_maps.append({
            "xT": xT, "xTo": xTo,
            "wq": Wq, "wk": Wk, "wv": Wv, "wo": Wo,
            "pet": pet_c.astype(bf),
            "bq": bqv, "bk": bkv, "bv": bvv, "bo": bov,
        })
    return in_maps


def kernel(x, W_qkv, b_qkv, pe, W_out, b_out, _trace=False):
    global _compiled
    if _compiled is None:
        _compiled = build_kernel()
    nc = _compiled
    in_maps = shard_inputs(x, W_qkv, b_qkv, pe, W_out, b_out)
    res = run_bass_kernel_spmd(nc, in_maps, core_ids=list(range(NCORES)),
                               trace=_trace)
    outs = res.results
    full = np.empty((B, S, E), np.float32)
    for c in range(NCORES):
        full[:, c * XB:(c + 1) * XB, :] = outs[c]["out"].reshape(B, XB, E)
    if _trace:
        kernel.last_exec_time_ns = res.exec_time_ns
        kernel.last_profile = res.profile_json
    return full

